# revision 1
# baseline (speedup 1.0000x reference)
"""DWT roundtrip (Haar wavedec2 x2 + band downsample -> cubic upsample + waverec2)
as a fused single-pass Trainium2 kernel.

Math: for input x, the reference computes
  aa1, lh1, hl1, hh1 = haar_dwt2(x)            # level-1 bands, [H/2, W/2]
  z = stack(haar_dwt2(aa1), area2(lh1), area2(hl1), area2(hh1))
  ...decode: aa1r = idwt2(dwt2(aa1)) == aa1 (exact roundtrip), and
  out = haar_idwt2(aa1, U(D(lh1)), U(D(hl1)), U(D(hh1)))
with U = cv2-cubic 2x upsample, D = 2x2 box mean. Everything is linear and
local, so the level-2 roundtrip cancels analytically and the whole model is

  out[2i+p, 2j+q] = P[i,j]/4 + UG_pq[i,j]        (p,q in {0,1})

where P = 2x2 block sums of x (== 2*aa1) and UG_pq = U_w(G_pq) with
  G_pq = (-1)^p DQ_lh + (-1)^q DQ_hl + (-1)^(p+q) DQ_hh,
DQ_b the 4x4-block Haar-detail sums of x (== 8*D(band_b)), and U_w the 2D cubic
upsample with the 1/16 normalization folded into its column matrix.

Layout: one image [512,512] per step; SBUF tile [128 partitions x 2048], each
partition owns 4 consecutive image rows, so every row/col pair op is a
free-axis DVE op. The quarter-res -> half-res cubic upsample runs on the
TensorEngine as Zt = G^T @ Ah^T (one matmul, no transposes needed) followed by
UG = (Zt-slice)^T @ Aw^T, and the P/4 term is accumulated into the same PSUM
via a 0.25*I matmul. ScalarE interleaves PSUM results into the output tile.

All elementwise work sits on the VectorE: GpSimd shares SBUF ports with it, so
splitting tensor_tensor work across the two just halves both engines' rates.
The aa-path (rs -> P) stays fp32; the detail path runs at DETAIL_DT (bf16
unlocks the DVE 2x packed mode; its rounding only touches the heavily-smoothed
detail bands, ~2e-4 relative).

Sharding: pure data-parallel, batch 32 -> 4 samples (12 images) per core.
"""

import numpy as np

import concourse.bass as bass
import concourse.mybir as mybir
from concourse import tile
from concourse.bass_utils import run_bass_kernel_spmd

N_CORES = 8
B, C, H, W = 32, 3, 512, 512
IMGS_PER_CORE = (B // N_CORES) * C  # 12

F32 = mybir.dt.float32
F32R = mybir.dt.float32r
BF16 = mybir.dt.bfloat16
ADD = mybir.AluOpType.add
SUB = mybir.AluOpType.subtract
AX = mybir.AxisListType.X

WE = (-0.03515625, 0.26171875, 0.87890625, -0.10546875)

# float32r streams 4x faster than float32 through the PE at N>=256 with
# near-fp32 accuracy (used for the aa-term identity matmuls and, when
# DETAIL_DT=F32, the upsample matmuls too).
MM_DT = F32R
DETAIL_DT = F32  # bf16 measured 4.6e-3 rel err (G-field ulp too coarse); F32 keeps 1.7e-4


def _build_A(n):
    """Cubic 2x upsample matrix [2n, n]: out = A @ q along an axis,
    edge-replicated like cv2 (weights accumulate on clamped taps)."""
    A = np.zeros((2 * n, n), dtype=np.float64)
    Wr = (WE[3], WE[2], WE[1], WE[0])
    for u in range(n):
        for t in range(4):
            A[2 * u, min(max(u - 2 + t, 0), n - 1)] += WE[t]
            A[2 * u + 1, min(max(u - 1 + t, 0), n - 1)] += Wr[t]
    return A


def _legalize_waits(nc):
    """This walrus build accepts at most one sync wait per instruction; Tile
    occasionally emits more (notably the kernel-tail DMA drain). Hoist extra
    waits onto standalone EventSemaphore instructions placed just before."""
    for f in nc.m.functions:
        for blk in f.blocks:
            new = []
            changed = False
            for inst in blk.instructions:
                si = inst.sync_info
                if si is not None and len(si.on_wait) > 1:
                    waits = list(si.on_wait)
                    for k, w in enumerate(waits[:-1]):
                        ev = mybir.InstEventSemaphore(
                            name=f"{inst.name}_hw{k}",
                            ins=[],
                            outs=[],
                            engine=inst.engine,
                            sync_info=mybir.SyncInfo(on_wait=[w], on_update=[]),
                        )
                        new.append(ev)
                    inst.sync_info = mybir.SyncInfo(
                        on_wait=[waits[-1]], on_update=list(si.on_update)
                    )
                    changed = True
                new.append(inst)
            if changed:
                blk.instructions = new


def build_nc(n_imgs=IMGS_PER_CORE, mm_dt=MM_DT, det_dt=DETAIL_DT, legalize=True):
    nc = bass.Bass(trn_type="TRN2", target_bir_lowering=False, debug=False)

    x = nc.dram_tensor("x", [n_imgs, H, W], F32, kind="ExternalInput").ap()
    y = nc.dram_tensor("y", [n_imgs, H, W], F32, kind="ExternalOutput").ap()

    # dtype of the upsample (G-band) matmuls follows the detail chain
    g_dt = det_dt if det_dt == BF16 else mm_dt

    A = _build_A(128)
    # AhT[k, n]: n<128 -> even half-rows A[2n,k]; n>=128 -> odd half-rows.
    AhT = np.concatenate([A[0::2, :].T, A[1::2, :].T], axis=1).astype(np.float32)
    AwT = (A.T / 16.0).astype(np.float32)  # [128, 256], natural col order
    np_g = mybir.dt.np(g_dt)
    ahT_d = nc.inline_tensor(np.ascontiguousarray(AhT.astype(np_g)), name="AhT").ap()
    awT_d = nc.inline_tensor(np.ascontiguousarray(AwT.astype(np_g)), name="AwT").ap()
    awTn_d = nc.inline_tensor(np.ascontiguousarray((-AwT).astype(np_g)), name="AwTn").ap()
    i4_d = nc.inline_tensor((0.25 * np.eye(128)).astype(np.float32), name="I4").ap()

    with tile.TileContext(nc) as tc:
        with (
            tc.tile_pool(name="const", bufs=1) as cpool,
            tc.tile_pool(name="io", bufs=4) as iop,
            tc.tile_pool(name="work", bufs=3) as wp,
            tc.tile_pool(name="psum", bufs=2, space="PSUM") as pzt,
            tc.tile_pool(name="psug", bufs=1, space="PSUM") as pug,
        ):
            ahT = cpool.tile([128, 256], g_dt, tag="ahT")
            awT = cpool.tile([128, 256], g_dt, tag="awT")
            awTn = cpool.tile([128, 256], g_dt, tag="awTn")
            i4 = cpool.tile([128, 128], F32, tag="i4")
            nc.sync.dma_start(out=ahT, in_=ahT_d.bitcast(g_dt))
            nc.sync.dma_start(out=awT, in_=awT_d.bitcast(g_dt))
            nc.sync.dma_start(out=awTn, in_=awTn_d.bitcast(g_dt))
            nc.sync.dma_start(out=i4.bitcast(mm_dt), in_=i4_d.bitcast(mm_dt))
            i4_r = i4.bitcast(mm_dt)

            def g_cast(ap):
                return ap if det_dt == BF16 else ap.bitcast(mm_dt)

            for m in range(n_imgs):
                # ---- load image: partition p <- rows 4p..4p+3 ----
                X = iop.tile([128, 2048], F32, tag="xin")
                nc.sync.dma_start(out=X, in_=x[m].rearrange("(p r) w -> p (r w)", p=128))
                X3 = X.rearrange("p (r w) -> p r w", r=4)

                # ---- aa path (fp32): rs = row pairs, P = 2x2 block sums ----
                RS = wp.tile([128, 1024], F32, tag="rs")
                RS3 = RS.rearrange("p (r w) -> p r w", r=2)
                nc.vector.tensor_tensor(out=RS3, in0=X3[:, 0::2, :], in1=X3[:, 1::2, :], op=ADD)
                P = wp.tile([128, 512], F32, tag="p")
                P3 = P.rearrange("p (r w) -> p r w", r=2)  # [128, 2, 256]
                nc.vector.tensor_tensor(
                    out=P3.bitcast(mm_dt), in0=RS3[:, :, 0::2], in1=RS3[:, :, 1::2], op=ADD
                )

                # ---- detail path (det_dt): e/o -> rss/rdd -> DQ bands ----
                # One TT makes both quarter-row cross sums:
                #   e = x[4u]+x[4u+2], o = x[4u+1]+x[4u+3]
                EO = wp.tile([128, 1024], det_dt, tag="eo")
                EO3 = EO.rearrange("p (r w) -> p r w", r=2)
                nc.vector.tensor_tensor(out=EO3, in0=X3[:, 0:2, :], in1=X3[:, 2:4, :], op=ADD)
                E, O = EO[:, 0:512], EO[:, 512:1024]
                # rss/rdd side by side so their column pair-diffs merge into one op
                RSD = wp.tile([128, 1024], det_dt, tag="rsd")
                nc.vector.tensor_tensor(out=RSD[:, 0:512], in0=E, in1=O, op=ADD)
                nc.vector.tensor_tensor(out=RSD[:, 512:1024], in0=E, in1=O, op=SUB)
                RSD3 = RSD.rearrange("p (r w) -> p r w", r=2)

                # column pair diffs of rss and rdd in one op: [q1 | c2]
                QC = wp.tile([128, 512], det_dt, tag="qc")
                QC3 = QC.rearrange("p (r w) -> p r w", r=2)
                nc.vector.tensor_tensor(
                    out=QC3, in0=RSD3[:, :, 0::2], in1=RSD3[:, :, 1::2], op=SUB
                )
                with nc.allow_low_precision(reason="smoothed detail bands tolerate 16-bit"):
                    # pair sums of [q1 | c2] -> [DQhl | DQhh]
                    DQ2 = wp.tile([128, 256], det_dt, tag="dq2")
                    nc.vector.tensor_reduce(
                        out=DQ2.rearrange("p (b v) -> p b v", b=2),
                        in_=QC.rearrange("p (b v k) -> p b v k", b=2, k=2),
                        axis=AX,
                        op=ADD,
                    )
                    DQhl, DQhh = DQ2[:, 0:128], DQ2[:, 128:256]

                # ---- Hadamard combos: one quarter-res field per output parity ----
                # Scratch layout T = [s | DQlh | d] lets the four G combos
                # collapse into two double-width ops:
                #   [G00|G11'] = [DQlh|DQlh] + [s|d]   (step-0 broadcast AP)
                #   [G01|G10]  = [DQlh|d] - [s|DQlh]   (overlapping slices)
                T = wp.tile([128, 384], det_dt, tag="t")
                Tr = T.rearrange("p (a w) -> p a w", a=3)
                nc.vector.tensor_tensor(out=Tr[:, 0, :], in0=DQhl, in1=DQhh, op=ADD)
                nc.vector.tensor_tensor(out=Tr[:, 2, :], in0=DQhl, in1=DQhh, op=SUB)
                DQlh = Tr[:, 1, :]  # reduce4 writes straight into the middle slot
                with nc.allow_low_precision(reason="smoothed detail bands tolerate 16-bit"):
                    nc.vector.tensor_reduce(
                        out=DQlh,
                        in_=RSD[:, 512:1024].rearrange("p (v k) -> p v k", k=4),
                        axis=AX,
                        op=ADD,
                    )
                import bass_rust as _br

                dql2 = _br.AP(
                    tensor=T.tensor,
                    offset=T.offset + 128,
                    ap=[list(T.ap[0]), [0, 2], [1, 128]],
                )
                GA = wp.tile([128, 256], det_dt, tag="ga")
                GA3 = GA.rearrange("p (a w) -> p a w", a=2)
                nc.vector.tensor_tensor(
                    out=g_cast(GA3), in0=dql2, in1=Tr[:, 0::2, :], op=ADD
                )
                GB = wp.tile([128, 256], det_dt, tag="gb")
                GB3 = GB.rearrange("p (a w) -> p a w", a=2)
                nc.vector.tensor_tensor(
                    out=g_cast(GB3), in0=Tr[:, 1:3, :], in1=Tr[:, 0:2, :], op=SUB
                )
                # bands b0..b3 = G00, G01, G10, G11' (b3 negated; AwTn compensates)
                G = [GA[:, 0:128], GB[:, 0:128], GB[:, 128:256], GA[:, 128:256]]

                # ---- cubic upsample on PE: Zt_b = G_b^T @ AhT  ([qcol, 256]) ----
                zts = []
                for pair in range(2):
                    zt_ps = pzt.tile([128, 512], F32, tag=f"zt{pair}")
                    for half in range(2):
                        bi = 2 * pair + half
                        nc.tensor.matmul(
                            out=zt_ps[:, half * 256 : half * 256 + 256],
                            lhsT=g_cast(G[bi]),
                            rhs=ahT,
                            start=True,
                            stop=True,
                        )
                    zt_sb = wp.tile([128, 512], g_dt, tag=f"ztsb{pair}")
                    nc.scalar.copy(out=g_cast(zt_sb) if det_dt != BF16 else zt_sb, in_=zt_ps)
                    zts.append(zt_sb)

                # ---- UG_pq = Zt-slice^T @ AwT (+ 0.25*I @ P) and interleave ----
                Xo = iop.tile([128, 2048], F32, tag="xout")
                Xo3 = Xo.rearrange("p (r w) -> p r w", r=4)
                for bi, (p_par, q_par) in enumerate([(0, 0), (0, 1), (1, 0), (1, 1)]):
                    zt_sb = zts[bi // 2]
                    zoff = (bi % 2) * 256
                    rhs = awTn if bi == 3 else awT
                    ug = pug.tile([128, 512], F32, tag=f"ug{bi}")
                    for par in range(2):  # half-row parity: ev, od
                        sl = slice(par * 256, par * 256 + 256)
                        lhsT = zt_sb[:, zoff + par * 128 : zoff + par * 128 + 128]
                        nc.tensor.matmul(
                            out=ug[:, sl],
                            lhsT=g_cast(lhsT),
                            rhs=rhs,
                            start=True,
                            stop=False,
                        )
                        nc.tensor.matmul(
                            out=ug[:, sl],
                            lhsT=i4_r,
                            rhs=P3[:, par, :].bitcast(mm_dt),
                            start=False,
                            stop=True,
                        )
                    ug3 = ug.rearrange("p (a b) -> p a b", a=2)
                    nc.scalar.copy(out=Xo3[:, p_par::2, q_par::2], in_=ug3)

                nc.sync.dma_start(out=y[m].rearrange("(p r) w -> p (r w)", p=128), in_=Xo)

    if legalize:
        _legalize_waits(nc)
    return nc


def kernel(x: np.ndarray) -> np.ndarray:
    x = np.ascontiguousarray(x, dtype=np.float32)
    assert x.shape == (B, C, H, W)
    nc = build_nc()
    per = B // N_CORES
    in_maps = [
        {"x": np.ascontiguousarray(x[i * per : (i + 1) * per].reshape(IMGS_PER_CORE, H, W))}
        for i in range(N_CORES)
    ]
    res = run_bass_kernel_spmd(nc, in_maps, core_ids=list(range(N_CORES)))
    out = np.empty((B, C, H, W), dtype=np.float32)
    for i in range(N_CORES):
        out[i * per : (i + 1) * per] = res.results[i]["y"].reshape(per, C, H, W)
    return out

